# revision 10
# baseline (speedup 1.0000x reference)
"""Trainium2 Bass kernel for nn_DGLayer (MMD-gated mixture of domain experts).

Reference computation:
    k_hv[b,m,n] = exp(-0.5*|h_b - v_mn|^2 / sigma^2)          (Gaussian kernel)
    kme_dot[b,m]  = mean_n k_hv                               <phi(h_b), mu_m>
    kme_norm2[m]  = mean_{n,n'} k(v_mn, v_mn')                |mu_m|^2
    mmd2[b,m]     = 1 - 2*kme_dot + kme_norm2
    prob          = softmax_m(-mmd2)
    out[b,u]      = sum_m prob[b,m] * (h @ W_m + b_m)[b,u]

Device strategy (data-parallel over 8 cores, batch sharded):
  The weighted expert sum is algebraically a single matmul with a mixed
  weight matrix when prob is batch-independent, and a K-concatenated
  matmul  X @ Wcat  (X = [prob[:,m] * h]_m, Wcat = [W_m]_m stacked on K)
  in the general case.  Both cases run the same tiled PSUM-accumulating
  matmul kernel; only K and the host-side operand prep differ.

  For the reference input distribution |h_b - v_mn|^2 >= ~850 always
  (E|h|^2 = D = 1024), so every k_hv underflows fp32 to exactly 0.0 and
  kme_dot is exactly 0 in the fp32 reference as well: prob reduces to
  softmax_m(-(1 + kme_norm2[m])) -- a function of the weights V only.
  We prove this per-call with a rigorous Cauchy-Schwarz bound
  (|h-v|^2 >= (|h|-|v|)^2) before taking the collapsed path; otherwise
  we fall back to the exact general path.
"""

import os

import numpy as np

# Problem shape (hardcoded per spec nn_DGLayer_25116968747262).
B, D, U, M_DOM, N_BASIS = 4096, 1024, 1024, 16, 64
N_CORES = 8
B_LOC = B // N_CORES  # 512 rows per core
SIGMA, SOFTNESS = 2.0, 1.0
GAMMA = -0.5 / (SIGMA * SIGMA)  # -0.125

P = 128          # SBUF partitions
FREE = 512       # matmul moving free dim (one PSUM bank of fp32)
BT = B_LOC // P  # 4 output row tiles
UT = U // FREE   # 2 output col tiles

# "float32r" streams 1 row/cycle on the PE (vs 4 for strict fp32) at
# slightly reduced multiplier precision; flip with env for experiments.
MM_DTYPE = os.environ.get("KERNEL_MM_DTYPE", "float32r")


CB = B_LOC + U  # concatenated free dim: [h^T | W] per K row


def _build_nc(k_total: int, mm_dtype: str):
    """Bass program: o[512,1024] = x[:, :512].T @ x[:, 512:], K=k_total.

    x packs the transposed activations and the weights side by side
    ([K, 512+1024] fp32) so each K-chunk group arrives in ONE DMA -- a
    PE/DMA instruction has a single ISA wait slot, so every instruction
    must depend on at most one semaphore.  HWDGE DMAs are tracked on 8
    round-robin lanes; reusing a lane adds a second (WAW) wait on the
    DMA, so the fast path issues at most 8 HWDGE DMAs total.  Large K
    falls back to SWDGE (gpsimd), whose single FIFO lane never needs a
    WAW wait.  The 4x2 grid of [128,512] output tiles accumulates in
    all 8 PSUM banks so the PE runs back-to-back matmuls for the whole
    K loop; evacuation is split DVE/ACT so the two halves drain in
    parallel into two output DMAs.
    """
    import concourse.tile as tile
    from concourse import bacc, mybir

    dt = getattr(mybir.dt, mm_dtype)
    kt = k_total // P

    if kt <= 8:
        # e.g. kt=8 -> [1,1,1,1,2,2]: 6 input + 2 output HWDGE DMAs.
        groups = [[k] for k in range(min(kt, 4))]
        if kt > 4:
            groups += [[4, 5], [6, 7]][:(kt - 4 + 1) // 2]
        use_hwdge = True
    else:
        groups = [list(range(g, min(g + 4, kt))) for g in range(0, kt, 4)]
        use_hwdge = False

    nc = bacc.Bacc("TRN2", target_bir_lowering=False, debug=False,
                   num_devices=N_CORES)
    x = nc.dram_tensor("x", [k_total, CB], dt, kind="ExternalInput").ap()
    o = nc.dram_tensor("o", [B_LOC, U], mybir.dt.float32,
                       kind="ExternalOutput").ap()

    x3 = x.rearrange("(ko p) c -> p ko c", p=P)
    o4 = o.rearrange("(oh bt p) u -> p oh bt u", p=P, oh=2)

    with tile.TileContext(nc) as tc:
        with (
            tc.tile_pool(name="xp", bufs=1) as xp,
            tc.tile_pool(name="op", bufs=1) as op,
            tc.tile_pool(name="ps", bufs=BT * UT, space="PSUM") as ps,
        ):
            psum = {}
            for b in range(BT):
                for u in range(UT):
                    psum[b, u] = ps.tile([P, FREE], mybir.dt.float32,
                                         tag="ps", name=f"ps_{b}_{u}")
            for gi, ks in enumerate(groups):
                ng = len(ks)
                nbufs = sum(len(g) == ng for g in groups) if use_hwdge else 3
                xg = xp.tile([P, ng, CB], dt, tag=f"x{ng}", bufs=nbufs,
                             name=f"x_{gi}")
                eng = nc.sync if use_hwdge else nc.gpsimd
                eng.dma_start(xg[:], x3[:, ks[0]:ks[0] + ng, :])
                for j, k in enumerate(ks):
                    for b in range(BT):
                        for u in range(UT):
                            nc.tensor.matmul(
                                psum[b, u][:],
                                xg[:, j, b * P:(b + 1) * P],
                                xg[:, j,
                                   B_LOC + u * FREE:B_LOC + (u + 1) * FREE],
                                start=(k == 0),
                                stop=(k == kt - 1),
                            )
            # Evacuate: half 0 (b=0,1) on DVE, half 1 (b=2,3) on ACT, so
            # each output DMA waits on exactly one engine semaphore.
            for half in range(2):
                ot = op.tile([P, 2, U], mybir.dt.float32, tag="o", bufs=2,
                             name=f"o_{half}")
                for bb in range(2):
                    for u in range(UT):
                        src = psum[half * 2 + bb, u][:]
                        dst = ot[:, bb, u * FREE:(u + 1) * FREE]
                        if half == 0:
                            nc.vector.tensor_copy(dst, src)
                        else:
                            nc.scalar.copy(dst, src)
                nc.sync.dma_start(o4[:, half], ot[:])
    nc.compile()
    return nc


def _install_ntff_hook():
    """Provide antenv.axon_hooks (absent in this container) so
    run_bass_kernel_spmd(trace=True) can capture NTFF profiles under
    axon.  Mirrors trn_agent_boot._ntff_profile_via_ctypes."""
    import contextlib
    import ctypes
    import sys
    import types

    if "antenv.axon_hooks" in sys.modules:
        return
    hook = None
    try:
        lib = ctypes.CDLL("/opt/axon/libaxon_pjrt.so")
        assert hasattr(lib, "axon_start_nrt_profile")
        lib.axon_start_nrt_profile.argtypes = [
            ctypes.POINTER(ctypes.c_int64), ctypes.c_size_t]
        lib.axon_start_nrt_profile.restype = ctypes.c_int64
        lib.axon_stop_nrt_profile.argtypes = [ctypes.c_char_p]
        lib.axon_stop_nrt_profile.restype = ctypes.c_int64

        @contextlib.contextmanager
        def _hook(output_dir, device_ids):
            import jax
            jax.devices()
            if device_ids:
                ids = (ctypes.c_int64 * len(device_ids))(*device_ids)
                rc = lib.axon_start_nrt_profile(ids, len(device_ids))
            else:
                rc = lib.axon_start_nrt_profile(None, 0)
            if rc != 0:
                raise RuntimeError(f"axon_start_nrt_profile rc={rc}")
            try:
                yield
            finally:
                n = lib.axon_stop_nrt_profile(str(output_dir).encode())
                print(f"ntff profile: {n} file(s) -> {output_dir}",
                      file=sys.stderr)

        hook = _hook
    except Exception:
        hook = None

    mod = types.ModuleType("antenv.axon_hooks")
    state = [hook]
    mod.get_axon_ntff_profile_hook = lambda: state[0]
    mod.set_axon_ntff_profile_hook = lambda h: state.__setitem__(0, h)
    sys.modules["antenv.axon_hooks"] = mod


def _run_device_matmul(ht_full: np.ndarray, w_full: np.ndarray,
                       **run_kwargs):
    """Run o = ht.T @ w on 8 cores, batch-sharded: core c gets
    ht[:, c*512:(c+1)*512].  Returns ([B, U] fp32, BassKernelResults)."""
    from concourse.bass_utils import run_bass_kernel_spmd

    if run_kwargs.get("trace"):
        _install_ntff_hook()

    k_total = ht_full.shape[0]
    nc = _build_nc(k_total, MM_DTYPE)
    in_maps = []
    for c in range(N_CORES):
        xc = np.empty((k_total, CB), dtype=np.float32)
        xc[:, :B_LOC] = ht_full[:, c * B_LOC:(c + 1) * B_LOC]
        xc[:, B_LOC:] = w_full
        in_maps.append({"x": xc})
    res = run_bass_kernel_spmd(nc, in_maps, core_ids=list(range(N_CORES)),
                               **run_kwargs)
    out = np.concatenate([r["o"] for r in res.results], axis=0)
    return out, res


def _kme_norm2(V: np.ndarray) -> np.ndarray:
    """mean_{n,n'} k(v_n, v_n') per domain, [M] fp64."""
    V64 = V.astype(np.float64)
    vn2 = np.einsum("mnd,mnd->mn", V64, V64)
    sq = vn2[:, :, None] + vn2[:, None, :] - 2.0 * np.einsum(
        "mnd,mkd->mnk", V64, V64)
    return np.exp(GAMMA * sq).mean(axis=(1, 2))


def _collapsed_probs(h: np.ndarray, V: np.ndarray):
    """If every cross kernel value k(h_b, v_mn) provably vanishes in fp32
    (so kme_dot is exactly 0 in the fp32 reference), return the
    batch-independent routing probs softmax_m(-(1+kme_norm2)); else None.
    """
    Vf = V.reshape(-1, V.shape[-1]).astype(np.float64)
    hn = np.linalg.norm(h.astype(np.float64), axis=1)
    vn = np.linalg.norm(Vf, axis=1)
    # |h-v|^2 >= (|h| - |v|)^2; underflow margin: need exp < 2^-25 to be
    # swamped by 1.0 in fp32; require < 1e-11 for slack.
    gap2_min = ((hn[:, None] - vn[None, :]) ** 2).min()
    if GAMMA * gap2_min > -25.3:
        return None
    logits = -SOFTNESS * (1.0 + _kme_norm2(V))
    x = logits - logits.max()
    p = np.exp(x)
    return p / p.sum()


def _exact_probs(h: np.ndarray, V: np.ndarray) -> np.ndarray:
    """Exact fp32 routing probs [B, M] (general fallback, host numpy)."""
    h32, V32 = h.astype(np.float32), V.astype(np.float32)
    Vf = V32.reshape(-1, D)
    sq = ((h32 * h32).sum(1, keepdims=True) - 2.0 * (h32 @ Vf.T)
          + (Vf * Vf).sum(1)[None, :])
    k_hv = np.exp(np.float32(GAMMA) * sq, dtype=np.float32)
    kme_dot = k_hv.reshape(B, M_DOM, N_BASIS).mean(-1)
    mmd2 = 1.0 - 2.0 * kme_dot + _kme_norm2(V32).astype(np.float32)[None, :]
    z = -SOFTNESS * mmd2
    z = z - z.max(axis=1, keepdims=True)
    e = np.exp(z)
    return (e / e.sum(axis=1, keepdims=True)).astype(np.float32)


def kernel(h, V, W, b, **run_kwargs):
    """Full-input entry point: h [4096,1024], V [16,64,1024],
    W [16,1024,1024], b [16,1024] -> [4096,1024] fp32."""
    h = np.ascontiguousarray(np.asarray(h, dtype=np.float32))
    V = np.ascontiguousarray(np.asarray(V, dtype=np.float32))
    W = np.ascontiguousarray(np.asarray(W, dtype=np.float32))
    b = np.ascontiguousarray(np.asarray(b, dtype=np.float32))

    p = _collapsed_probs(h, V)
    if p is not None:
        # prob is batch-independent: out = h @ (sum_m p_m W_m) + p @ b.
        w_mix = np.einsum("m,mdu->du", p, W.astype(np.float64))
        b_mix = (p @ b.astype(np.float64)).astype(np.float32)
        out, res = _run_device_matmul(
            np.ascontiguousarray(h.T), w_mix.astype(np.float32), **run_kwargs)
    else:
        # General path: out[b] = sum_m prob[b,m] * (h @ W_m) + prob[b] @ b.
        # One K=M*D matmul of X = [prob[:,m]*h]_m against stacked W.
        probs = _exact_probs(h, V)
        x = (probs.T[:, :, None] * h[None, :, :])       # [M, B, D]
        xt = np.ascontiguousarray(
            x.transpose(0, 2, 1).reshape(M_DOM * D, B), dtype=np.float32)
        wcat = np.ascontiguousarray(W.reshape(M_DOM * D, U))
        b_mix = (probs @ b).astype(np.float32)
        out, res = _run_device_matmul(xt, wcat, **run_kwargs)

    out = out + b_mix
    kernel.last_results = res
    return out.astype(np.float32)
